# revision 49
# baseline (speedup 1.0000x reference)
"""DyGraphGIN2d Trainium kernel: kNN graph (k=16) + GIN aggregation + MLP/BN/GELU.

Sharding: data-parallel over batch B=8 across 8 NeuronCores (one batch
element per core). BatchNorm uses training-mode batch statistics over ALL
B*N rows, so per-core partial sums go through one in-kernel AllReduce.

Algorithm per core (N=4096 points, C=64 dims), single-matmul phases:
  The ranking metric s[n,m] = <x_hi_n, x_hi_m> + q_m with q = -|x|^2/2
  carried EXACTLY as two f32r rank-1 rows (q_hi + q_lo, an exact hi/lo
  split), all folded into ONE K=66 f32r matmul per 128x512 block (extra
  contraction rows are free: matmul cost is output-columns only).
  Phase 1 (tau): per 128-row stripe, 8 such matmuls + DVE top-8 per
  512-chunk + max/match_replace/max give the 16th-largest s per row;
  tau rides back into the XL operand as two more exact f32r rank-1 rows
  (-tau_hi, -tau_lo).
  Phase 2 (mask+aggregate): v' = s^T - tau is recomputed transposed by the
  mirrored K=68 matmul -- the first 66 product terms are bitwise identical
  to phase 1 (commuted multiplies, same PSUM order), so selection is
  bit-consistent; the 1e-5 guard inside tau makes v' > 0 strict for the
  16 selected neighbors. Masks {0,1} are made off the DVE: ACT computes
  sigmoid(4e6 * v') from PSUM (saturates to exactly 1.0/0.0 in f32r);
  the final chunk's masks alternate onto the then-idle DVE (is_gt).
  GPSIMD cannot touch PSUM, so it only gets SBUF-side prologue work.
  aggr[c,n] accumulates in PSUM via single-pass f32r matmuls with x_hi^T
  (PE-transposed).
  The phase-1 stripes (DVE-bound) and phase-2 blocks (PE-bound) are
  software-pipelined chunk-by-chunk so PE/DVE/ACT/Pool all stay busy.
  The (1+eps)*x_hi self term rides the same PSUM via one diag(1+eps)
  matmul per chunk, so "finish" is a single ACT PSUM->SBUF copy.
  Tail: h1 = W1^T h + b1 (f32r PE + ACT bias w/ accum_out BN sums); BN
  stats AllGather + local 8-way add (the collective cost is ~15us fixed,
  AllReduce would be 1.875x); fused BN+erf-GELU on ACT; out = W2^T hg+b2.

The jitted 8-core shard_map executable is cached across kernel() calls.
"""

import numpy as np

import concourse.bacc as bacc

import concourse.mybir as mybir
from concourse.tile import TileContext

F32 = mybir.dt.float32
F32R = mybir.dt.float32r
AF = mybir.ActivationFunctionType
ALU = mybir.AluOpType

B, C, N, O = 8, 64, 4096, 64
K_NN = 16
N_CORES = 8
NT = N // 128          # 32 row stripes
NCH = 8                # 512-wide column chunks
BN_EPS = 1e-5
BN_COUNT = float(B * N)
TAU_GUARD = 1e-5
SIG_SCALE = 4e6

_cache = {}


def _build():
    nc = bacc.Bacc("TRN2", target_bir_lowering=False)

    xb_d = nc.dram_tensor("xb", [C, N], F32, kind="ExternalInput")
    w1_d = nc.dram_tensor("w1", [C, O], F32, kind="ExternalInput")
    w2_d = nc.dram_tensor("w2", [O, O], F32, kind="ExternalInput")
    vecs_d = nc.dram_tensor("vecs", [O, 5], F32, kind="ExternalInput")  # b1,gamma,beta,b2,eps1
    ones2_d = nc.dram_tensor("ones2", [2, N], F32R, kind="ExternalInput")
    identr_d = nc.dram_tensor("identr", [C, C], F32R, kind="ExternalInput")
    y_d = nc.dram_tensor("y", [O, N], F32, kind="ExternalOutput")
    tau_scr = nc.dram_tensor("tau_scr", [N, 2], F32R)  # internal scratch

    with TileContext(nc) as tc:
        with tc.tile_pool(name="big", bufs=1) as big, \
             tc.tile_pool(name="work", bufs=1) as work, \
             tc.tile_pool(name="dram", bufs=1, space="DRAM") as dpool:

            # ---------------- prologue: operands ----------------
            vecs_sb = work.tile([O, 5], F32)
            w1_sb = work.tile([C, O], F32)
            w2_sb = work.tile([O, O], F32)
            identr = work.tile([C, C], F32R)
            w1r = work.tile([C, O], F32R)
            w2r = work.tile([O, O], F32R)
            eid = work.tile([C, C], F32R)

            xbc = [big.tile([C, 512], F32, name=f"xbc{i}") for i in range(NCH)]
            # XLc: p1 lhsT rows [x_hi; 1; 1], p2 rhs rows [x_hi; 1; 1;
            # -tau_hi; -tau_lo].  The tau rows are DMA-written only after all
            # p1 stripes of the chunk have been emitted, so no false
            # whole-tile WAR stalls arise.
            XLc = [big.tile([128, 512], F32R, name=f"XLc{i}") for i in range(NCH)]
            XRc = [big.tile([128, 512], F32R, name=f"XRc{i}") for i in range(NCH)]
            xt_sb = big.tile([128, NT * C], F32R)

            import concourse.bass_isa as bass_isa
            xsq = work.tile([C, 512], F32, tag="xsq", bufs=2)
            sqall = work.tile([C, 512], F32, tag="sqall", bufs=2)
            qt = work.tile([1, 512], F32, tag="qt", bufs=2)
            # xb loads first: they head the per-chunk critical chains and the
            # single HWDGE queue drains in emission order.
            for c in range(NCH):
                nc.sync.dma_start(xbc[c][:, :], xb_d[:, c * 512 : (c + 1) * 512])
            nc.sync.dma_start(vecs_sb[:, :], vecs_d[:, :])
            nc.sync.dma_start(w1_sb[:, :], w1_d[:, :])
            nc.sync.dma_start(w2_sb[:, :], w2_d[:, :])
            nc.sync.dma_start(identr[:, :], identr_d[:, :])
            nc.scalar.activation(w1r[:, :], w1_sb[:, :], AF.Copy)
            nc.scalar.activation(w2r[:, :], w2_sb[:, :], AF.Copy)
            # diag(1+eps) in f32r: folds the (1+eps)*x_hi term into the
            # aggregation PSUM via one extra matmul per chunk.
            nc.scalar.activation(eid[:, :], identr[:, :], AF.Copy,
                                 scale=vecs_sb[:, 4:5])

            for c in range(NCH):
                sl = slice(c * 512, (c + 1) * 512)
                nc.scalar.activation(XLc[c][:C, :], xbc[c][:, :], AF.Copy)
                nc.scalar.dma_start(XLc[c][C : C + 2, :], ones2_d[:, sl])
                nc.gpsimd.tensor_copy(XRc[c][:C, :], XLc[c][:C, :])
                nc.scalar.dma_start(XRc[c][C + 2 : C + 4, :], ones2_d[:, sl])
                # q = -|x_m|^2/2 exactly as f32r hi+lo rank-1 rows; the
                # partition reduce runs on the idle GPSIMD engine (cold-PE
                # fp32 matmuls here cost ~2.4us each at low p-state).
                xsq_t = work.tile([C, 512], F32, tag="xsq", bufs=2, name=f"xsq_{c}")
                sq_t = work.tile([C, 512], F32, tag="sqall", bufs=2, name=f"sqa_{c}")
                nc.gpsimd.tensor_tensor(out=xsq_t[:, :], in0=xbc[c][:, :],
                                        in1=xbc[c][:, :], op=ALU.mult)
                nc.gpsimd.partition_all_reduce(sq_t[:, :], xsq_t[:, :],
                                               channels=C,
                                               reduce_op=bass_isa.ReduceOp.add)
                nc.scalar.activation(XRc[c][C : C + 1, :], sq_t[0:1, :], AF.Copy,
                                     scale=-0.5)
                qh_t = work.tile([1, 512], F32R, tag="qh", bufs=2, name=f"qh_{c}")
                nc.scalar.activation(qh_t[:, :], sq_t[0:1, :], AF.Copy, scale=-0.5)
                qt_t = work.tile([1, 512], F32, tag="qt", bufs=2, name=f"qt_{c}")
                nc.vector.tensor_scalar(out=qt_t[:, :], in0=sq_t[0:1, :],
                                        scalar1=-0.5, scalar2=None, op0=ALU.mult)
                nc.vector.tensor_tensor(out=qt_t[:, :], in0=qt_t[:, :],
                                        in1=qh_t.bitcast(F32)[:, :],
                                        op=ALU.subtract)
                # ACT can only write at partition base 0/64; q_lo (row 65)
                # goes through a partition-0 staging tile + DMA.
                ql_t = work.tile([1, 512], F32R, tag="ql", bufs=2,
                                 name=f"ql_{c}")
                nc.scalar.activation(ql_t[:, :], qt_t[:, :], AF.Copy)
                nc.sync.dma_start(XRc[c][C + 1 : C + 2, :], ql_t[:, :])

            # ---------------- main pipelined loop ----------------
            # ps_v/ps_a are entered only after the transpose block below so
            # its PSUM fits; ps_s is needed from iteration 0.
            ps_s_cm = tc.tile_pool(name="ps_s", bufs=3, space="PSUM")
            ps_s = ps_s_cm.__enter__()
            ps_v = ps_a = None
            _cms = [ps_s_cm]

            cand = work.tile([128, 64], F32, tag="cand", bufs=5)
            t8a = work.tile([128, 8], F32, tag="t8a", bufs=4)
            t8b = work.tile([128, 8], F32, tag="t8b", bufs=4)
            ntf = work.tile([128, 1], F32, tag="ntf", bufs=4)
            ntau2 = work.tile([128, 2], F32R, tag="ntau2", bufs=4)
            mask = work.tile([128, 512], F32R, tag="mask", bufs=6)
            hc = work.tile([C, 512], F32R, tag="hc", bufs=3)
            h1c = [big.tile([O, 512], F32, name=f"h1c{i}") for i in range(NCH)]
            bnsum = work.tile([O, NCH], F32)
            bnsq = work.tile([O, NCH], F32)
            sqscr = work.tile([O, 512], F32, tag="sqscr", bufs=2)
            eps1 = vecs_sb[:, 4:5]

            aggr_tiles = {}

            def p1_mm(s, c8, cand_t):
                jt, jo = s // 4, (s % 4) * 128
                s_ps = ps_s.tile([128, 512], F32, tag="s_ps", name=f"s_{s}_{c8}")
                nc.tensor.matmul(s_ps[:, :], XLc[jt][: C + 2, jo : jo + 128],
                                 XRc[c8][: C + 2, :], start=True, stop=True)
                nc.vector.max(out=cand_t[:, c8 * 8 : (c8 + 1) * 8], in_=s_ps[:, :])

            def p1_tail(s, cand_t):
                jt, jo = s // 4, (s % 4) * 128
                t8a_t = work.tile([128, 8], F32, tag="t8a", bufs=4, name=f"t8a_{s}")
                t8b_t = work.tile([128, 8], F32, tag="t8b", bufs=4, name=f"t8b_{s}")
                ntf_t = work.tile([128, 1], F32, tag="ntf", bufs=4, name=f"ntf_{s}")
                nt2_t = work.tile([128, 2], F32R, tag="ntau2", bufs=4, name=f"nt2_{s}")
                nc.vector.max(out=t8a_t[:, :], in_=cand_t[:, :])
                nc.vector.match_replace(out=cand_t[:, :], in_to_replace=t8a_t[:, :],
                                        in_values=cand_t[:, :], imm_value=-1e30)
                nc.vector.max(out=t8b_t[:, :], in_=cand_t[:, :])
                # -tau = -(t16 - guard) = guard - t16, split exactly hi+lo
                nc.gpsimd.tensor_scalar(out=ntf_t[:, :], in0=t8b_t[:, 7:8],
                                        scalar1=-1.0, scalar2=TAU_GUARD,
                                        op0=ALU.mult, op1=ALU.add)
                nc.vector.tensor_copy(nt2_t[:, 0:1], ntf_t[:, :])
                nc.gpsimd.tensor_tensor(out=nt2_t.bitcast(F32)[:, 1:2],
                                        in0=ntf_t[:, :],
                                        in1=nt2_t.bitcast(F32)[:, 0:1],
                                        op=ALU.subtract)
                # SBUF->SBUF DMA cannot transpose partition->free; bounce
                # the per-stripe [128,2] tau pair through flat DRAM.  The
                # chunk-wide readback into XLc happens in tau_readback().
                nc.sync.dma_start(tau_scr[s * 128 : (s + 1) * 128, :], nt2_t[:, 0:2])

            def p2_v(c, j):
                jt, jo = j // 4, (j % 4) * 128
                v_ps = ps_v.tile([128, 512], F32, tag="v_ps", name=f"v_{c}_{j}")
                nc.tensor.matmul(v_ps[:, :], XRc[jt][: C + 4, jo : jo + 128],
                                 XLc[c][: C + 4, :], start=True, stop=True)
                m = work.tile([128, 512], F32R, tag="mask", bufs=6, name=f"m_{c}_{j}")
                if c == NCH - 1 and j % 2 == 0:
                    nc.vector.tensor_scalar(out=m[:, :], in0=v_ps[:, :],
                                            scalar1=0.0, scalar2=None,
                                            op0=ALU.is_gt)
                else:
                    nc.scalar.activation(m[:, :], v_ps[:, :], AF.Sigmoid,
                                         scale=SIG_SCALE)
                return m

            def p2_aggr(c, j, m):
                nc.tensor.matmul(aggr_tiles[c][:, :],
                                 xt_sb[:, j * C : (j + 1) * C],
                                 m[:, :],
                                 start=(j == 0), stop=False)

            def finish_mlp(c):
                sl = slice(c * 512, (c + 1) * 512)
                nc.tensor.matmul(aggr_tiles[c][:, :], eid[:, :], XLc[c][:C, :],
                                 start=False, stop=True)
                h_t = work.tile([C, 512], F32R, tag="hc", bufs=3, name=f"h_{c}")
                nc.scalar.activation(h_t[:, :], aggr_tiles[c][:, :], AF.Copy)
                h1_ps = ps_v.tile([O, 512], F32, tag="v_ps", name=f"h1ps_{c}")
                nc.tensor.matmul(h1_ps[:, :], w1r[:, :], h_t[:, :],
                                 start=True, stop=True)
                nc.scalar.activation(h1c[c][:, :], h1_ps[:, :], AF.Identity,
                                     bias=vecs_sb[:, 0:1],
                                     accum_out=bnsum[:, c : c + 1])
                sq_t = work.tile([O, 512], F32, tag="sqscr", bufs=2, name=f"sq_{c}")
                nc.scalar.activation(sq_t[:, :], h1c[c][:, :], AF.Square,
                                     accum_out=bnsq[:, c : c + 1])

            # software pipeline: iteration it runs phase-1 stripes of chunk
            # it and phase-2 of chunk it-1, interleaved 1:1 on the PE stream.
            for it in range(NCH + 1):
                if it == 1:
                    # x_hi^T chunks for the aggregation matmuls: emitted here
                    # so the PE transposes + ACT copies overlap iteration 0's
                    # DVE-only top-8 work.
                    with tc.tile_pool(name="ps_tp", bufs=2, space="PSUM") as ps_tp:
                        for j in range(NT):
                            tp = ps_tp.tile([128, C], F32R, tag="tp_ps")
                            nc.tensor.transpose(
                                tp[:, :],
                                XLc[j // 4][:C, (j % 4) * 128 : (j % 4 + 1) * 128],
                                identr[:, :])
                            nc.scalar.activation(xt_sb[:, j * C : (j + 1) * C],
                                                 tp[:, :], AF.Copy)
                    ps_v_cm = tc.tile_pool(name="ps_v", bufs=3, space="PSUM")
                    ps_a_cm = tc.tile_pool(name="ps_a", bufs=2, space="PSUM")
                    ps_v = ps_v_cm.__enter__()
                    ps_a = ps_a_cm.__enter__()
                    _cms.extend([ps_v_cm, ps_a_cm])
                c1 = it if it < NCH else None
                c2 = it - 1 if it >= 1 else None
                if c2 is not None:
                    aggr_tiles[c2] = ps_a.tile([O, 512], F32, tag="aggr",
                                               name=f"aggr_{c2}")
                cands = {}
                masks = {}
                if c1 is not None:
                    for s in range(4 * c1, 4 * c1 + 4):
                        cands[s] = work.tile([128, 64], F32, tag="cand",
                                             bufs=5, name=f"cand_{s}")
                for k in range(NT):
                    if c1 is not None:
                        if c2 is None:
                            # iteration 0: chunk-major order so the early
                            # chunks' top-8 passes run while the later
                            # prologue chunks are still being prepared.
                            s, c8 = 4 * c1 + k % 4, k // 4
                        else:
                            s, c8 = 4 * c1 + k // 8, k % 8
                        p1_mm(s, c8, cands[s])
                    if c2 is not None:
                        masks[k] = p2_v(c2, k)
                        if k >= 2:
                            p2_aggr(c2, k - 2, masks.pop(k - 2))
                if c1 is not None:
                    # tails only after every stripe's XLc reads are emitted:
                    # the tau DMAs then order cleanly behind them.
                    for kk in range(4):
                        s = 4 * c1 + kk
                        p1_tail(s, cands[s])
                    nc.sync.dma_start(
                        XLc[c1][C + 2 : C + 4, :],
                        tau_scr[c1 * 512 : (c1 + 1) * 512, 0:2].rearrange(
                            "p two -> two p"))
                if c2 is not None:
                    p2_aggr(c2, NT - 2, masks.pop(NT - 2))
                    p2_aggr(c2, NT - 1, masks.pop(NT - 1))
                    finish_mlp(c2)

            # ---------------- BN combine + AllReduce + GELU + W2 ---------
            stats = work.tile([O, 2], F32)
            nc.vector.reduce_sum(stats[:, 0:1], bnsum[:, :], axis=mybir.AxisListType.X)
            nc.vector.reduce_sum(stats[:, 1:2], bnsq[:, :], axis=mybir.AxisListType.X)

            # AllGather + local 8-way add: the collective cost model charges
            # AllReduce 1.875x the fixed ~15us latency, AllGather 1x.
            cc_in = dpool.tile([O, 2], F32)
            cc_out = dpool.tile([N_CORES * O, 2], F32, addr_space="Shared")
            nc.sync.dma_start(cc_in[:, :], stats[:, :])
            nc.gpsimd.collective_compute(
                "AllGather", ALU.bypass,
                ins=[cc_in[:, :]],
                outs=[cc_out[:, :]],
                replica_groups=[list(range(N_CORES))],
            )
            gall = work.tile([O, 2 * N_CORES], F32)
            nc.sync.dma_start(
                gall[:, :].rearrange("p (two k) -> p two k", two=2, k=N_CORES),
                cc_out[:, :].rearrange("(k p) two -> p two k", k=N_CORES, p=O))
            gstats = work.tile([O, 2], F32)
            nc.vector.reduce_sum(
                gstats[:, 0:2],
                gall[:, :].rearrange("p (two k) -> p two k", two=2, k=N_CORES),
                axis=mybir.AxisListType.X)

            var = work.tile([O, 1], F32)
            scale = work.tile([O, 1], F32)
            shift = work.tile([O, 1], F32)
            tmp = work.tile([O, 1], F32)
            mv = work.tile([O, 2], F32)
            nc.vector.tensor_scalar(out=mv[:, :], in0=gstats[:, 0:2],
                                    scalar1=1.0 / BN_COUNT, scalar2=None, op0=ALU.mult)
            mean = mv[:, 0:1]
            var_t = mv[:, 1:2]
            nc.vector.tensor_tensor(out=tmp[:, :], in0=mean, in1=mean,
                                    op=ALU.mult)
            nc.vector.tensor_tensor(out=var[:, :], in0=var_t, in1=tmp[:, :],
                                    op=ALU.subtract)
            nc.vector.tensor_scalar(out=var[:, :], in0=var[:, :], scalar1=BN_EPS,
                                    scalar2=None, op0=ALU.add)
            nc.scalar.activation(tmp[:, :], var[:, :], AF.Sqrt)
            nc.vector.reciprocal(out=tmp[:, :], in_=tmp[:, :])
            nc.vector.tensor_tensor(out=scale[:, :], in0=vecs_sb[:, 1:2],
                                    in1=tmp[:, :], op=ALU.mult)
            nc.vector.tensor_tensor(out=tmp[:, :], in0=mean, in1=scale[:, :],
                                    op=ALU.mult)
            nc.vector.tensor_tensor(out=shift[:, :], in0=vecs_sb[:, 2:3],
                                    in1=tmp[:, :], op=ALU.subtract)

            hg = work.tile([O, 512], F32R, tag="hg", bufs=3)
            y_sb = work.tile([O, 512], F32, tag="ysb", bufs=3)
            for c in range(NCH):
                sl = slice(c * 512, (c + 1) * 512)
                hg_t = work.tile([O, 512], F32R, tag="hg", bufs=3, name=f"hg_{c}")
                nc.scalar.activation(hg_t[:, :], h1c[c][:, :], AF.Gelu,
                                     scale=scale[:, :], bias=shift[:, :])
                o_ps = ps_v.tile([O, 512], F32, tag="v_ps", name=f"ops_{c}")
                nc.tensor.matmul(o_ps[:, :], w2r[:, :], hg_t[:, :],
                                 start=True, stop=True)
                y_t = work.tile([O, 512], F32, tag="ysb", bufs=3, name=f"y_{c}")
                nc.vector.tensor_scalar(out=y_t[:, :], in0=o_ps[:, :],
                                        scalar1=vecs_sb[:, 3:4], scalar2=None,
                                        op0=ALU.add)
                nc.sync.dma_start(y_d[:, sl], y_t[:, :])

            for cm in reversed(_cms):
                cm.__exit__(None, None, None)

    if not nc.is_finalized():
        nc.finalize()
    return nc


def _get_runner():
    """Build the Bass module once and cache a jitted 8-core executable."""
    if "runner" in _cache:
        return _cache["runner"]

    import jax
    import concourse.mybir as mb
    from jax.sharding import Mesh, PartitionSpec
    from jax.experimental.shard_map import shard_map
    from concourse import bass2jax

    nc = _build()
    bass2jax.install_neuronx_cc_hook()

    partition_name = nc.partition_id_tensor.name if nc.partition_id_tensor else None
    in_names = []
    out_names = []
    out_avals = []
    for alloc in nc.m.functions[0].allocations:
        if not isinstance(alloc, mb.MemoryLocationSet):
            continue
        name = alloc.memorylocations[0].name
        if alloc.kind == "ExternalInput":
            if name != partition_name:
                in_names.append(name)
        elif alloc.kind == "ExternalOutput":
            out_names.append(name)
            out_avals.append(
                jax.core.ShapedArray(tuple(alloc.tensor_shape), mb.dt.np(alloc.dtype))
            )
    n_params = len(in_names)
    all_in_names = list(in_names)
    if partition_name is not None:
        all_in_names = all_in_names + [partition_name]

    def _body(*args):
        operands = list(args)
        if partition_name is not None:
            operands.append(bass2jax.partition_id_tensor())
        outs = bass2jax._bass_exec_p.bind(
            *operands,
            out_avals=tuple(out_avals),
            in_names=tuple(all_in_names),
            out_names=tuple(out_names),
            lowering_input_output_aliases=(),
            sim_require_finite=True,
            sim_require_nnan=True,
            nc=nc,
        )
        return tuple(outs)

    devices = jax.devices()[:N_CORES]
    assert len(devices) == N_CORES, f"need {N_CORES} devices, have {len(jax.devices())}"
    mesh = Mesh(np.asarray(devices), ("core",))
    n_outs = len(out_names)
    sharded = jax.jit(
        shard_map(
            _body,
            mesh=mesh,
            in_specs=(PartitionSpec("core"),) * n_params,
            out_specs=(PartitionSpec("core"),) * n_outs,
            check_rep=False,
        ),
        keep_unused=True,
    )
    _cache["runner"] = (sharded, in_names, out_names, out_avals)
    return _cache["runner"]


def kernel(**inputs) -> np.ndarray:
    x = np.asarray(inputs["x"], dtype=np.float32)
    assert x.shape == (B, C, N, 1), x.shape
    k = int(np.asarray(inputs.get("k", K_NN)))
    assert k == K_NN, f"kernel compiled for k={K_NN}, got {k}"
    w1 = np.asarray(inputs["w1"], dtype=np.float32)
    b1 = np.asarray(inputs["b1"], dtype=np.float32)
    gamma = np.asarray(inputs["gamma"], dtype=np.float32)
    beta = np.asarray(inputs["beta"], dtype=np.float32)
    w2 = np.asarray(inputs["w2"], dtype=np.float32)
    b2 = np.asarray(inputs["b2"], dtype=np.float32)
    eps_gin = float(np.asarray(inputs["eps_gin"]))

    sharded, in_names, out_names, out_avals = _get_runner()

    xb = np.ascontiguousarray(x[:, :, :, 0])                     # [B, C, N]
    vecs = np.stack(
        [b1, gamma, beta, b2, np.full(O, 1.0 + eps_gin, np.float32)], axis=1
    ).astype(np.float32)                                         # [64, 5]
    ones2 = np.ones((2, N), np.float32)
    ones_col = np.ones((C, 1), np.float32)
    identr = np.eye(C, dtype=np.float32)

    per_core = {
        "xb": xb,
        "w1": np.broadcast_to(w1, (N_CORES,) + w1.shape),
        "w2": np.broadcast_to(w2, (N_CORES,) + w2.shape),
        "vecs": np.broadcast_to(vecs, (N_CORES,) + vecs.shape),
        "ones2": np.broadcast_to(ones2, (N_CORES,) + ones2.shape),
        "ones_col": np.broadcast_to(ones_col, (N_CORES,) + ones_col.shape),
        "identr": np.broadcast_to(identr, (N_CORES,) + identr.shape),
    }
    concat_in = [
        np.ascontiguousarray(per_core[name]).reshape(
            (N_CORES * per_core[name].shape[1],) + per_core[name].shape[2:]
        )
        for name in in_names
    ]
    out_arrs = sharded(*concat_in)
    yi = out_names.index("y")
    y = np.asarray(out_arrs[yi]).reshape(N_CORES, O, N)
    return y[..., None].astype(np.float32)
